# revision 79
# baseline (speedup 1.0000x reference)
"""Top-1 MoE block (B=4, S=2048, H=2048, E=8) for 8 Trainium2 NeuronCores.

Strategy (expert-parallel, host-mediated dispatch):
  - Host computes the tiny gating network (x @ Wg -> softmax -> argmax),
    0.4% of total FLOPs, and the token permutation per expert.
  - Token block for expert e (prob-scaled, cast to bf16, tiled) plus
    W[e] (bf16) goes to core e.  Each core runs a dense matmul in bf16
    (full PE rate, half the HBM traffic of fp32).
  - Tokens beyond 1024 per expert ("overflow", ~210 of 8192 here) are
    packed into one extra half-contraction tile per core: a core pair
    (2g, 2g+1) computes K-halves [0:1024) / [1024:2048) of overflow
    group g; the host sums the two partial outputs.  This keeps every
    core at 8 full m-tiles + 1 half-K tile instead of 9 full tiles.
  - Host upcasts per-expert outputs, scatters back to token order.

Schedule per core (all numbers warm-clock):
  - The first 256 tokens (m-tiles 0,1) are packed as four quarter-K
    blocks (xq) interleaved into the two HWDGE queues with the W
    k-tiles, ordered so PE consumption (~1.7us per 0.5MB k-tile at
    8 matmuls/tile) tracks the ~200GB/s-per-queue delivery curve.
  - Tensor engine pre-warms the PE (HAM un-throttle) with junk matmuls
    until the first blocks land, then runs k-major over m-tiles {0,1},
    then streams the remaining m-tiles with W resident.
  - m-tile 0 is copied out of PSUM in four nt-slices and m-tile 2 runs
    nt-major gated per-slice, so the PSUM handoff has no bubble.
  - The final (overflow) tile runs nt-major so its casts+stores
    pipeline behind the last matmuls (vector nt0/1/3, ACT nt2).
"""

import os

import numpy as np
import ml_dtypes

import concourse.bass as bass
from concourse import mybir
from concourse.bass_utils import run_bass_kernel_spmd

BF16 = ml_dtypes.bfloat16

B, S, H, E = 4, 2048, 2048, 8
P = 128
KT = H // P  # 16 k tiles
N_FREE = 512  # matmul moving free dim / PSUM bank width (fp32)
NT = H // N_FREE  # 4 n tiles
N_CORES = 8
# final-tile col slices: 3x512 then 2x256 (short tail); index 2 is ACT's
F_SLICES = [(0, 512), (512, 512), (1024, 512), (1536, 512)]
# final-tile copy/store granules: vector+sync handle F_COPIES, ACT
# (scalar) handles F_ACT — the last 512 is split 2x256 across both
# pipes so the two store chains drain in parallel
F_COPIES = [(0, 0, 512), (1, 512, 512), (3, 1536, 256)]
F_ACT = [(2, 1024, 512), (3, 1792, 256)]
N_WARM = 32  # pre-warm matmuls, N=256 (~213ns cold / ~110ns warm each)
WARM_N = 256
CAP = 1024  # main-tile token capacity per core in overflow mode

_COMPILED = {}


def _ensure_ntff_hook() -> bool:
    """Register antenv.axon_hooks with a ctypes NTFF hook if the image lacks it."""
    import contextlib
    import ctypes
    import sys
    import types

    try:
        from antenv.axon_hooks import get_axon_ntff_profile_hook  # noqa: F401

        return True
    except ImportError:
        pass

    so_path = "/opt/axon/libaxon_pjrt.so"
    if not os.path.exists(so_path):
        return False
    lib = ctypes.CDLL(so_path)
    if not hasattr(lib, "axon_start_nrt_profile"):
        return False
    lib.axon_start_nrt_profile.argtypes = [
        ctypes.POINTER(ctypes.c_int64),
        ctypes.c_size_t,
    ]
    lib.axon_start_nrt_profile.restype = ctypes.c_int64
    lib.axon_stop_nrt_profile.argtypes = [ctypes.c_char_p]
    lib.axon_stop_nrt_profile.restype = ctypes.c_int64

    @contextlib.contextmanager
    def _hook(output_dir, device_ids):
        import jax

        jax.devices()  # force PJRT init so the .so's client exists
        if device_ids:
            ids = (ctypes.c_int64 * len(device_ids))(*device_ids)
            rc = lib.axon_start_nrt_profile(ids, len(device_ids))
        else:
            rc = lib.axon_start_nrt_profile(None, 0)
        if rc != 0:
            raise RuntimeError(f"axon_start_nrt_profile rc={rc}")
        try:
            yield
        finally:
            n = lib.axon_stop_nrt_profile(str(output_dir).encode())
            print(f"ntff profile: {n} file(s) -> {output_dir}")

    import antenv

    mod = types.ModuleType("antenv.axon_hooks")
    mod.get_axon_ntff_profile_hook = lambda: _hook
    mod.set_axon_ntff_profile_hook = lambda h: None
    sys.modules["antenv.axon_hooks"] = mod
    antenv.axon_hooks = mod
    return True


def _build_bass(n_main: int, ov: bool) -> bass.Bass:
    """SPMD kernel for one core.

    Main tiles: y[mt] = x[mt].T @ w for mt in 0..n_main-1 (full K=2048).
    Final tile: ov=True  -> y2 = xt2.T @ w2 with K=1024 (overflow half).
                ov=False -> one more main m-tile (full K), nt-major.

    xq:  [512, 1024] four quarter-K blocks of m-tiles 0,1:
         row q*128+p, col kl*256+t = x_token[t][(4q+kl)*128+p], t in
         0..255 spanning both m-tiles.
    xt:  [(MTx-2)*128, 2048] per-m-tile transposed blocks for tiles 2..:
         row (mt-2)*128+p, col kt*128+t = x_token[mt*128+t][kt*128+p].
    w:   [H, H] row-major.  y: [MTx*128, H] bf16.
    xt2: [128, 1024], w2: [1024, H], y2: [128, H] (ov mode only).
    """
    assert n_main >= 4
    f32 = mybir.dt.float32
    bf16 = mybir.dt.bfloat16
    MTx = n_main if ov else n_main + 1  # m-tiles in y
    # ov: the overflow slot holds ONE <=32-token group whose K=2048 is
    # split across the 4 PE column-groups (4 col-tiled matmuls run
    # concurrently per 512-cycle span) -> 4x4 spans instead of 8x4.
    KTF = KT if ov else KT  # final-tile W k-tiles (full W either way)
    HH = H // 2

    nc = bass.Bass()
    xq = nc.dram_tensor("xq", [4 * P, 4 * 2 * P], bf16, kind="ExternalInput")
    xt = nc.dram_tensor(
        "xt", [(MTx - 2) * P, KT * P], bf16, kind="ExternalInput"
    )
    w = nc.dram_tensor("w", [H, H], bf16, kind="ExternalInput")
    y = nc.dram_tensor("y", [MTx * P, H], bf16, kind="ExternalOutput")
    if ov:
        xt2 = nc.dram_tensor("xt2", [P, KT * 32], bf16, kind="ExternalInput")
        w2 = nc.dram_tensor("w2", [H, H], bf16, kind="ExternalInput")
        y2 = nc.dram_tensor("y2", [P, H], bf16, kind="ExternalOutput")

    with (
        nc.sbuf_tensor("w_sb", [P, KT, H], bf16) as w_sb,
        nc.sbuf_tensor("x01_sb", [P, KT, 2 * P], bf16) as x01_sb,
        nc.sbuf_tensor("x_sb", [P, n_main - 2, H], bf16) as x_sb,
        nc.sbuf_tensor("y_sb", [P, n_main, H], bf16) as y_sb,
        nc.sbuf_tensor("xf_sb", [P, (KT * 32) if ov else (KT * P)], bf16)
        as xf_sb,
        nc.sbuf_tensor("yf_sb", [P, H], bf16) as yf_sb,
        nc.sbuf_tensor(
            "wf_sb", [P, KTF if ov else 1, H if ov else 2], bf16
        ) as wf_alloc,
        nc.sbuf_tensor("warm", [P, WARM_N], bf16) as warm,
        nc.psum_tensor("ps0", [P, H], f32) as ps0,
        nc.psum_tensor("ps1", [P, H], f32) as ps1,
        nc.semaphore("sPE") as sPE,
        nc.semaphore("sCopy") as sCopy,
        nc.semaphore("sWarm") as sWarm,
        nc.semaphore("sXf") as sXf,
        nc.semaphore("sCLv") as sCLv,
        nc.semaphore("sCLg") as sCLg,
        nc.semaphore("sYsync") as sYsync,
        nc.semaphore("sYscal") as sYscal,
        nc.semaphore("sXQ0b") as sXQ0b,
        nc.Block() as block,
    ):
        psums = [ps0, ps1]
        sW = [nc.semaphore(f"sW{kt}").__enter__() for kt in range(KT)]
        sXQ = [nc.semaphore(f"sXQ{q}").__enter__() for q in range(4)]
        sX = [nc.semaphore(f"sX{mt}").__enter__() for mt in range(2, n_main)]
        sY = [nc.semaphore(f"sY{mt}").__enter__() for mt in range(n_main)]
        ps_f = psums[n_main % 2]
        if ov:
            wf_sb = wf_alloc
            sWf = [nc.semaphore(f"sWf{kt}").__enter__() for kt in range(KTF)]
        else:
            wf_sb, sWf = w_sb, sW  # final tile reuses resident W

        def w_dma(eng, kt, half):
            # each W k-tile is split across both queues; sW[kt] reaches 32
            # when both halves landed.  One slow queue then only bends the
            # arrival curve instead of stalling every other k-tile.
            c0 = half * HH
            eng.dma_start(
                w_sb[:, kt, c0 : c0 + HH], w[kt * P : (kt + 1) * P, c0 : c0 + HH]
            ).then_inc(sW[kt], 16)

        def xq_dma(eng, q):
            eng.dma_start(
                x01_sb[:, 4 * q : 4 * (q + 1), :], xq[q * P : (q + 1) * P, :]
            ).then_inc(sXQ[q], 16)

        def x_dma(eng, mt):
            eng.dma_start(
                x_sb[:, mt - 2, :], xt[(mt - 2) * P : (mt - 1) * P, :]
            ).then_inc(sX[mt - 2], 16)

        yf_dst = y2 if ov else y
        r0 = 0 if ov else n_main * P

        @block.sync
        def _(sync):
            # interleaved head: xq quarters + W0a + even W k-tiles, then
            # late x tiles, even w2 tiles, final stores 0,1,3
            xq_dma(sync, 0)
            for kt in range(KT):
                w_dma(sync, kt, 0)
                if kt == 2:
                    xq_dma(sync, 1)
                elif kt == 6:
                    xq_dma(sync, 3)
            for mt in range(4, n_main):
                x_dma(sync, mt)
            if ov:
                for kt in range(0, KTF, 2):
                    sync.dma_start(
                        wf_sb[:, kt, :], w2[kt * P : (kt + 1) * P, :]
                    ).then_inc(sWf[kt], 16)
            for n_vs, (i, c0, cw) in enumerate(F_COPIES, start=1):
                sync.wait_ge(sCLv, n_vs)
                sync.dma_start(
                    yf_dst[r0 : r0 + P, c0 : c0 + cw],
                    yf_sb[:, c0 : c0 + cw],
                ).then_inc(sYsync, 16)
            sync.wait_ge(sYsync, 16 * len(F_COPIES))

        @block.scalar
        def _(scalar):
            # W0b, W1, xq2, odd W k-tiles, x2/x3, xf, odd w2 tiles,
            # main stores, final store 2
            for kt in range(KT):
                w_dma(scalar, kt, 1)
                if kt == 5:
                    xq_dma(scalar, 2)
            x_dma(scalar, 2)
            x_dma(scalar, 3)
            if ov:
                scalar.dma_start(xf_sb[:, :], xt2[:, :]).then_inc(sXf, 16)
                for kt in range(1, KTF, 2):
                    scalar.dma_start(
                        wf_sb[:, kt, :], w2[kt * P : (kt + 1) * P, :]
                    ).then_inc(sWf[kt], 16)
            else:
                scalar.dma_start(
                    xf_sb[:, :], xt[(n_main - 2) * P : (n_main - 1) * P, :]
                ).then_inc(sXf, 16)
            for mt in range(n_main):
                scalar.wait_ge(sCopy, mt + 4 if mt else 4)
                scalar.dma_start(
                    y[mt * P : (mt + 1) * P, :], y_sb[:, mt, :]
                ).then_inc(sY[mt], 16)
            # ACT casts its own final slice then stores it; the sem wait
            # orders the DMA behind the copy's SBUF writes (same-engine
            # issue does NOT imply write completion).
            for n_act, (i, c0, cw) in enumerate(F_ACT, start=1):
                scalar.wait_ge(sPE, n_main + 4 + i)
                scalar.copy(
                    yf_sb[:, c0 : c0 + cw], ps_f[:, c0 : c0 + cw]
                ).then_inc(sCLg, 1)
                scalar.wait_ge(sCLg, n_act)
                scalar.dma_start(
                    yf_dst[r0 : r0 + P, c0 : c0 + cw], yf_sb[:, c0 : c0 + cw]
                ).then_inc(sYscal, 16)
            for mt in range(n_main):
                scalar.wait_ge(sY[mt], 16)
            scalar.wait_ge(sYscal, 16 * len(F_ACT))

        @block.tensor
        def _(tensor):
            def mm01(psum, mt, kt, nt, start, stop):
                return tensor.matmul(
                    psum[:, nt * N_FREE : (nt + 1) * N_FREE],
                    x01_sb[:, kt, mt * P : (mt + 1) * P],
                    w_sb[:, kt, nt * N_FREE : (nt + 1) * N_FREE],
                    start=start,
                    stop=stop,
                    skip_group_check=True,
                )

            def mm(psum, mt, kt, nt, start, stop):
                return tensor.matmul(
                    psum[:, nt * N_FREE : (nt + 1) * N_FREE],
                    x_sb[:, mt - 2, kt * P : (kt + 1) * P],
                    w_sb[:, kt, nt * N_FREE : (nt + 1) * N_FREE],
                    start=start,
                    stop=stop,
                    skip_group_check=True,
                )

            # Pre-warm the PE (HAM un-throttles after ~3.4us of activity)
            # on scratch data while the first DMAs land.
            tensor.wait_ge(sWarm, 1)
            for _ in range(N_WARM):
                tensor.matmul(
                    ps0[:, 0:WARM_N],
                    warm[:, 0:P],
                    warm[:, :],
                    start=True,
                    stop=True,
                    skip_group_check=True,
                )

            # Phase 1: m-tiles 0,1 k-major chasing the W/xq DMA streams.
            for kt in range(KT):
                if kt % 4 == 0:
                    tensor.wait_ge(sXQ[kt // 4], 16)
                if kt == 0:
                    # bridge the xq0->W0 arrival window with small junk
                    # matmuls: free on typical cores (they idle here) and
                    # on a late-queue core they extend PE activity so the
                    # HAM window doesn't re-throttle before the stream.
                    for _ in range(20):
                        tensor.matmul(
                            ps0[:, 0:P],
                            warm[:, 0:P],
                            warm[:, 0:P],
                            start=True,
                            stop=True,
                            skip_group_check=True,
                        )
                tensor.wait_ge(sW[kt], 32)
                last = kt == KT - 1
                for nt in range(NT):
                    m = mm01(ps0, 0, kt, nt, kt == 0, last)
                    if last:
                        m.then_inc(sPE, 1)  # per-nt: mt0 copy pipelines
                for nt in range(NT):
                    m = mm01(ps1, 1, kt, nt, kt == 0, last)
                if last:
                    m.then_inc(sPE, 1)
            # Phase 2 head: m-tile 2 nt-major, gated on m-tile 0's
            # per-slice PSUM copies (no handoff bubble).
            tensor.wait_ge(sX[0], 16)
            for nt in range(NT):
                tensor.wait_ge(sCopy, nt + 1)
                for kt in range(KT):
                    m = mm(ps0, 2, kt, nt, kt == 0, kt == KT - 1)
            m.then_inc(sPE, 1)
            # Phase 2: W resident; stream the remaining m-tiles.
            for mt in range(3, n_main):
                tensor.wait_ge(sX[mt - 2], 16)
                tensor.wait_ge(sCopy, mt + 2)  # psum slot free
                for kt in range(KT):
                    for nt in range(NT):
                        m = mm(psums[mt % 2], mt, kt, nt, kt == 0, kt == KT - 1)
                m.then_inc(sPE, 1)
            # Final tile: col-slice-major so each slice finishes early and
            # its cast+store pipelines behind the remaining matmuls; the
            # last 512 is split in two 256s to shorten the very tail.
            tensor.wait_ge(sXf, 16)
            tensor.wait_ge(sCopy, n_main + 2)
            if ov:
                for kt in range(KTF):
                    tensor.wait_ge(sWf[kt], 16)
            for i, (c0, cw) in enumerate(F_SLICES):
                if ov:
                    # 4 col-tiled jobs per span: col-group q contracts the
                    # group's K-quarter q into psum rows 32q..32q+32; the
                    # four matmuls stream concurrently (own XBUS each).
                    for kl in range(4):
                        for q in range(4):
                            kt = q * 4 + kl
                            m = tensor.matmul(
                                ps_f[32 * q : 32 * (q + 1), c0 : c0 + cw],
                                xf_sb[:, kt * 32 : (kt + 1) * 32],
                                wf_sb[:, kt, c0 : c0 + cw],
                                start=(kl == 0),
                                stop=(kl == 3),
                                skip_group_check=True,
                                tile_position=(0, 32 * q),
                            )
                else:
                    for kt in range(KTF):
                        if i == 0:
                            tensor.wait_ge(sWf[kt], 32)
                        m = tensor.matmul(
                            ps_f[:, c0 : c0 + cw],
                            xf_sb[:, kt * P : (kt + 1) * P],
                            wf_sb[:, kt, c0 : c0 + cw],
                            start=(kt == 0),
                            stop=(kt == KTF - 1),
                            skip_group_check=True,
                        )
                m.then_inc(sPE, 1)

        @block.vector
        def _(vector):
            vector.memset(warm[:, :], 0.25).then_inc(sWarm, 1)
            # m-tile 0 in nt-slices (pipelines with mt1 kt15 + mt2 nt-major)
            for nt in range(NT):
                vector.wait_ge(sPE, nt + 1)
                vector.tensor_copy(
                    y_sb[:, 0, nt * N_FREE : (nt + 1) * N_FREE],
                    ps0[:, nt * N_FREE : (nt + 1) * N_FREE],
                ).then_inc(sCopy, 1)
            for mt in range(1, n_main):
                vector.wait_ge(sPE, mt + 4)
                vector.tensor_copy(
                    y_sb[:, mt, :], psums[mt % 2][:, :]
                ).then_inc(sCopy, 1)
            for i, c0, cw in F_COPIES:
                vector.wait_ge(sPE, n_main + 4 + i)
                vector.tensor_copy(
                    yf_sb[:, c0 : c0 + cw], ps_f[:, c0 : c0 + cw]
                ).then_inc(sCLv, 1)

    return nc


def _route(x, Wg):
    """Host gating: returns token indices per expert and top-1 probs."""
    xf = np.ascontiguousarray(x.reshape(-1, H))
    logits = xf @ Wg  # [T, E] fp32 (min top1-top2 gap ~1e-4)
    idx = logits.argmax(-1)
    m = logits.max(-1, keepdims=True)
    ex = np.exp(logits - m)
    p = (ex[np.arange(len(idx)), idx] / ex.sum(-1)).astype(np.float32)
    return xf, idx, p


def _pack_tiles(xs: np.ndarray, n_tiles: int, k: int, t0_tok: int = 0):
    """tokens [t0_tok + mt*128 + t] -> [n_tiles*128, k] bf16 tiles.

    Row mt*128+p, col kt*128+t  <-  xs[t0_tok + mt*128+t, kt*128+p].
    """
    n = xs.shape[0]
    kt = k // P
    out = np.zeros((n_tiles * P, k), dtype=BF16)
    for mt in range(n_tiles):
        t0, t1 = t0_tok + mt * P, min(t0_tok + (mt + 1) * P, n)
        if t0 >= t1:
            break
        blk = xs[t0:t1].astype(BF16)  # [tc, k]
        tc = t1 - t0
        dst = out[mt * P : (mt + 1) * P].reshape(P, kt, P)  # [p, kt, t]
        dst[:, :, :tc] = blk.reshape(tc, kt, P).transpose(2, 1, 0)
    return out


def _pack_xq(xs: np.ndarray) -> np.ndarray:
    """First 256 tokens -> [512, 1024] quarter-K blocks (see _build_bass)."""
    blk = np.zeros((2 * P, H), dtype=BF16)
    n = min(xs.shape[0], 2 * P)
    blk[:n] = xs[:n].astype(BF16)
    a = blk.reshape(2 * P, KT, P).transpose(1, 2, 0)  # [kt, p, t]
    out = np.empty((4 * P, 4 * 2 * P), dtype=BF16)
    for q in range(4):
        out[q * P : (q + 1) * P] = (
            a[4 * q : 4 * (q + 1)].transpose(1, 0, 2).reshape(P, 4 * 2 * P)
        )
    return out


def _run(inputs, trace=False):
    x = np.asarray(inputs["x"], dtype=np.float32)
    Wg = np.asarray(inputs["Wg"], dtype=np.float32)
    W = np.asarray(inputs["W"], dtype=np.float32)
    b = np.asarray(inputs["b"], dtype=np.float32)

    if trace:
        trace = _ensure_ntff_hook()

    xf, idx, p = _route(x, Wg)
    T = xf.shape[0]

    toks = [np.nonzero(idx == e)[0] for e in range(E)]
    counts = np.array([len(t) for t in toks])

    # Overflow groups: per-expert token chunks beyond CAP, each <= 32
    # (one col-tiled overflow slot per core).  If there are more groups
    # than cores, the smallest go to the host (<=0.1% of FLOPs here).
    groups = []
    for e in range(E):
        o = toks[e][CAP:]
        for i in range(0, len(o), 32):
            groups.append((e, o[i : i + 32]))
    groups.sort(key=lambda g: -len(g[1]))
    host_groups = groups[N_CORES:]
    groups = groups[:N_CORES]

    ov = len(groups) > 0
    if ov:
        n_main = CAP // P
        key = ("OV", n_main)
    else:
        n_main = max(4, int(-(-counts.max() // P)) - 1)
        key = ("A", n_main)
    if key not in _COMPILED:
        _COMPILED[key] = _build_bass(n_main, ov)
    nc = _COMPILED[key]

    MTx = n_main if ov else n_main + 1
    Wbf = [W[e].astype(BF16) for e in range(E)]
    in_maps = []
    for c in range(N_CORES):
        e = c
        te = toks[e][: CAP if ov else None]
        xs = xf[te] * p[te, None]  # fold gate prob into activations
        m = {
            "xq": _pack_xq(xs),
            "xt": _pack_tiles(xs, MTx - 2, H, t0_tok=2 * P),
            "w": Wbf[e],
        }
        if ov:
            if c < len(groups):
                e2, t2 = groups[c]
                xs2 = (xf[t2] * p[t2, None]).astype(BF16)  # [tc<=32, H]
                tc = len(t2)
                xt2 = np.zeros((P, KT * 32), dtype=BF16)
                # col block kt holds k-rows kt*128..: xt2[p, kt*32+t]
                a = xs2.reshape(tc, KT, P).transpose(1, 2, 0)  # [kt, p, t]
                xt2.reshape(P, KT, 32)[:, :, :tc] = a.transpose(1, 0, 2)
                m["xt2"] = xt2
                m["w2"] = Wbf[e2]
            else:
                m["xt2"] = np.zeros((P, KT * 32), dtype=BF16)
                m["w2"] = np.zeros((H, H), dtype=BF16)
        in_maps.append(m)

    res = run_bass_kernel_spmd(
        nc,
        in_maps,
        core_ids=list(range(N_CORES)),
        trace=trace,
        trace_cores=list(range(N_CORES)) if trace else None,
    )

    out = np.empty((T, H), dtype=np.float32)
    for e in range(E):
        te = toks[e][: CAP if ov else None]
        ye = res.results[e]["y"][: len(te)].astype(np.float32)
        if np.any(b[e]):
            ye = ye + p[te, None] * b[e]
        out[te] = ye
    if ov:
        for g, (e2, t2) in enumerate(groups):
            tc = len(t2)
            y2 = res.results[g]["y2"]
            ye = sum(
                y2[32 * q : 32 * q + tc].astype(np.float32) for q in range(4)
            )
            if np.any(b[e2]):
                ye = ye + p[t2, None] * b[e2]
            out[t2] = ye
        for e2, t2 in host_groups:
            ye = (p[t2, None] * (xf[t2] @ W[e2])).astype(np.float32)
            if np.any(b[e2]):
                ye = ye + p[t2, None] * b[e2]
            out[t2] = ye
    return out.reshape(B, S, H), res


def kernel(**inputs) -> np.ndarray:
    out, _ = _run(inputs, trace=os.environ.get("MOE_TRACE", "0") == "1")
    return out


def run_traced(inputs):
    """For test.py: returns (output, BassKernelResults with exec_time_ns)."""
    return _run(inputs, trace=True)


# revision 80
# speedup vs baseline: 1.0429x; 1.0429x over previous
"""Top-1 MoE block (B=4, S=2048, H=2048, E=8) for 8 Trainium2 NeuronCores.

Strategy (expert-parallel, host-mediated dispatch):
  - Host computes the tiny gating network (x @ Wg -> softmax -> argmax),
    0.4% of total FLOPs, and the token permutation per expert.
  - Token block for expert e (prob-scaled, cast to bf16, tiled) plus
    W[e] (bf16) goes to core e.  Each core runs a dense matmul in bf16
    (full PE rate, half the HBM traffic of fp32).
  - Tokens beyond 1024 per expert ("overflow", ~210 of 8192 here) are
    packed into one extra half-contraction tile per core: a core pair
    (2g, 2g+1) computes K-halves [0:1024) / [1024:2048) of overflow
    group g; the host sums the two partial outputs.  This keeps every
    core at 8 full m-tiles + 1 half-K tile instead of 9 full tiles.
  - Host upcasts per-expert outputs, scatters back to token order.

Schedule per core (all numbers warm-clock):
  - The first 256 tokens (m-tiles 0,1) are packed as four quarter-K
    blocks (xq) interleaved into the two HWDGE queues with the W
    k-tiles, ordered so PE consumption (~1.7us per 0.5MB k-tile at
    8 matmuls/tile) tracks the ~200GB/s-per-queue delivery curve.
  - Tensor engine pre-warms the PE (HAM un-throttle) with junk matmuls
    until the first blocks land, then runs k-major over m-tiles {0,1},
    then streams the remaining m-tiles with W resident.
  - m-tile 0 is copied out of PSUM in four nt-slices and m-tile 2 runs
    nt-major gated per-slice, so the PSUM handoff has no bubble.
  - The final (overflow) tile runs nt-major so its casts+stores
    pipeline behind the last matmuls (vector nt0/1/3, ACT nt2).
"""

import os

import numpy as np
import ml_dtypes

import concourse.bass as bass
from concourse import mybir
from concourse.bass_utils import run_bass_kernel_spmd

BF16 = ml_dtypes.bfloat16

B, S, H, E = 4, 2048, 2048, 8
P = 128
KT = H // P  # 16 k tiles
N_FREE = 512  # matmul moving free dim / PSUM bank width (fp32)
NT = H // N_FREE  # 4 n tiles
N_CORES = 8
# final-tile col slices: 3x512 then 2x256 (short tail); index 2 is ACT's
F_SLICES = [(0, 512), (512, 512), (1024, 512), (1536, 512)]
# final-tile copy/store granules: vector+sync handle F_COPIES, ACT
# (scalar) handles F_ACT — the last 512 is split 2x256 across both
# pipes so the two store chains drain in parallel
F_COPIES = [(0, 0, 512), (1, 512, 512), (3, 1536, 256)]
F_ACT = [(2, 1024, 512), (3, 1792, 256)]
N_WARM = 32  # pre-warm matmuls, N=256 (~213ns cold / ~110ns warm each)
WARM_N = 256
CAP = 1024  # main-tile token capacity per core in overflow mode

_COMPILED = {}


def _ensure_ntff_hook() -> bool:
    """Register antenv.axon_hooks with a ctypes NTFF hook if the image lacks it."""
    import contextlib
    import ctypes
    import sys
    import types

    try:
        from antenv.axon_hooks import get_axon_ntff_profile_hook  # noqa: F401

        return True
    except ImportError:
        pass

    so_path = "/opt/axon/libaxon_pjrt.so"
    if not os.path.exists(so_path):
        return False
    lib = ctypes.CDLL(so_path)
    if not hasattr(lib, "axon_start_nrt_profile"):
        return False
    lib.axon_start_nrt_profile.argtypes = [
        ctypes.POINTER(ctypes.c_int64),
        ctypes.c_size_t,
    ]
    lib.axon_start_nrt_profile.restype = ctypes.c_int64
    lib.axon_stop_nrt_profile.argtypes = [ctypes.c_char_p]
    lib.axon_stop_nrt_profile.restype = ctypes.c_int64

    @contextlib.contextmanager
    def _hook(output_dir, device_ids):
        import jax

        jax.devices()  # force PJRT init so the .so's client exists
        if device_ids:
            ids = (ctypes.c_int64 * len(device_ids))(*device_ids)
            rc = lib.axon_start_nrt_profile(ids, len(device_ids))
        else:
            rc = lib.axon_start_nrt_profile(None, 0)
        if rc != 0:
            raise RuntimeError(f"axon_start_nrt_profile rc={rc}")
        try:
            yield
        finally:
            n = lib.axon_stop_nrt_profile(str(output_dir).encode())
            print(f"ntff profile: {n} file(s) -> {output_dir}")

    import antenv

    mod = types.ModuleType("antenv.axon_hooks")
    mod.get_axon_ntff_profile_hook = lambda: _hook
    mod.set_axon_ntff_profile_hook = lambda h: None
    sys.modules["antenv.axon_hooks"] = mod
    antenv.axon_hooks = mod
    return True


def _build_bass(n_main: int, ov: bool) -> bass.Bass:
    """SPMD kernel for one core.

    Main tiles: y[mt] = x[mt].T @ w for mt in 0..n_main-1 (full K=2048).
    Final tile: ov=True  -> y2 = xt2.T @ w2 with K=1024 (overflow half).
                ov=False -> one more main m-tile (full K), nt-major.

    xq:  [512, 1024] four quarter-K blocks of m-tiles 0,1:
         row q*128+p, col kl*256+t = x_token[t][(4q+kl)*128+p], t in
         0..255 spanning both m-tiles.
    xt:  [(MTx-2)*128, 2048] per-m-tile transposed blocks for tiles 2..:
         row (mt-2)*128+p, col kt*128+t = x_token[mt*128+t][kt*128+p].
    w:   [H, H] row-major.  y: [MTx*128, H] bf16.
    xt2: [128, 1024], w2: [1024, H], y2: [128, H] (ov mode only).
    """
    assert n_main >= 4
    f32 = mybir.dt.float32
    bf16 = mybir.dt.bfloat16
    MTx = n_main if ov else n_main + 1  # m-tiles in y
    # ov: the overflow slot holds ONE <=32-token group whose K=2048 is
    # split across the 4 PE column-groups (4 col-tiled matmuls run
    # concurrently per 512-cycle span) -> 4x4 spans instead of 8x4.
    KTF = KT if ov else KT  # final-tile W k-tiles (full W either way)
    HH = H // 2

    nc = bass.Bass()
    xq = nc.dram_tensor("xq", [4 * P, 4 * 2 * P], bf16, kind="ExternalInput")
    xt = nc.dram_tensor(
        "xt", [(MTx - 2) * P, KT * P], bf16, kind="ExternalInput"
    )
    w = nc.dram_tensor("w", [H, H], bf16, kind="ExternalInput")
    y = nc.dram_tensor("y", [MTx * P, H], bf16, kind="ExternalOutput")
    if ov:
        xt2 = nc.dram_tensor("xt2", [P, KT * 32], bf16, kind="ExternalInput")
        w2 = nc.dram_tensor("w2", [H, H], bf16, kind="ExternalInput")
        y2 = nc.dram_tensor("y2", [P, H], bf16, kind="ExternalOutput")

    with (
        nc.sbuf_tensor("w_sb", [P, KT, H], bf16) as w_sb,
        nc.sbuf_tensor("x01_sb", [P, KT, 2 * P], bf16) as x01_sb,
        nc.sbuf_tensor("x_sb", [P, n_main - 2, H], bf16) as x_sb,
        nc.sbuf_tensor("y_sb", [P, n_main, H], bf16) as y_sb,
        nc.sbuf_tensor("xf_sb", [P, (KT * 32) if ov else (KT * P)], bf16)
        as xf_sb,
        nc.sbuf_tensor("yf_sb", [P, H], bf16) as yf_sb,
        nc.sbuf_tensor(
            "wf_sb", [P, KTF if ov else 1, H if ov else 2], bf16
        ) as wf_alloc,
        nc.sbuf_tensor("warm", [P, WARM_N], bf16) as warm,
        nc.psum_tensor("ps0", [P, H], f32) as ps0,
        nc.psum_tensor("ps1", [P, H], f32) as ps1,
        nc.semaphore("sPE") as sPE,
        nc.semaphore("sCopy") as sCopy,
        nc.semaphore("sWarm") as sWarm,
        nc.semaphore("sXf") as sXf,
        nc.semaphore("sCLv") as sCLv,
        nc.semaphore("sCLg") as sCLg,
        nc.semaphore("sYsync") as sYsync,
        nc.semaphore("sYscal") as sYscal,
        nc.semaphore("sXQ0b") as sXQ0b,
        nc.Block() as block,
    ):
        psums = [ps0, ps1]
        sW = [nc.semaphore(f"sW{kt}").__enter__() for kt in range(KT)]
        sXQ = [nc.semaphore(f"sXQ{q}").__enter__() for q in range(4)]
        sX = [nc.semaphore(f"sX{mt}").__enter__() for mt in range(2, n_main)]
        sY = [nc.semaphore(f"sY{mt}").__enter__() for mt in range(n_main)]
        ps_f = psums[n_main % 2]
        if ov:
            wf_sb = wf_alloc
            sWf = [nc.semaphore(f"sWf{kt}").__enter__() for kt in range(KTF)]
        else:
            wf_sb, sWf = w_sb, sW  # final tile reuses resident W

        def w_dma(eng, kt, half):
            # each W k-tile is split across both queues; sW[kt] reaches 32
            # when both halves landed.  One slow queue then only bends the
            # arrival curve instead of stalling every other k-tile.
            c0 = half * HH
            eng.dma_start(
                w_sb[:, kt, c0 : c0 + HH], w[kt * P : (kt + 1) * P, c0 : c0 + HH]
            ).then_inc(sW[kt], 16)

        def xq_dma(eng, q):
            eng.dma_start(
                x01_sb[:, 4 * q : 4 * (q + 1), :], xq[q * P : (q + 1) * P, :]
            ).then_inc(sXQ[q], 16)

        def x_dma(eng, mt):
            eng.dma_start(
                x_sb[:, mt - 2, :], xt[(mt - 2) * P : (mt - 1) * P, :]
            ).then_inc(sX[mt - 2], 16)

        yf_dst = y2 if ov else y
        r0 = 0 if ov else n_main * P

        @block.sync
        def _(sync):
            # interleaved head: xq quarters + W0a + even W k-tiles, then
            # late x tiles, even w2 tiles, final stores 0,1,3
            xq_dma(sync, 0)
            for kt in range(KT):
                w_dma(sync, kt, 0)
                if kt == 2:
                    xq_dma(sync, 1)
                elif kt == 6:
                    xq_dma(sync, 3)
            for mt in range(4, n_main):
                x_dma(sync, mt)
            if ov:
                for kt in range(0, KTF, 2):
                    sync.dma_start(
                        wf_sb[:, kt, :], w2[kt * P : (kt + 1) * P, :]
                    ).then_inc(sWf[kt], 16)
            for n_vs, (i, c0, cw) in enumerate(F_COPIES, start=1):
                sync.wait_ge(sCLv, n_vs)
                sync.dma_start(
                    yf_dst[r0 : r0 + P, c0 : c0 + cw],
                    yf_sb[:, c0 : c0 + cw],
                ).then_inc(sYsync, 16)
            sync.wait_ge(sYsync, 16 * len(F_COPIES))

        @block.scalar
        def _(scalar):
            # W0b, W1, xq2, odd W k-tiles, x2/x3, xf, odd w2 tiles,
            # main stores, final store 2
            for kt in range(KT):
                w_dma(scalar, kt, 1)
                if kt == 5:
                    xq_dma(scalar, 2)
            x_dma(scalar, 2)
            x_dma(scalar, 3)
            if ov:
                scalar.dma_start(xf_sb[:, :], xt2[:, :]).then_inc(sXf, 16)
                for kt in range(1, KTF, 2):
                    scalar.dma_start(
                        wf_sb[:, kt, :], w2[kt * P : (kt + 1) * P, :]
                    ).then_inc(sWf[kt], 16)
            else:
                scalar.dma_start(
                    xf_sb[:, :], xt[(n_main - 2) * P : (n_main - 1) * P, :]
                ).then_inc(sXf, 16)
            for mt in range(n_main):
                scalar.wait_ge(sCopy, mt + 4 if mt else 4)
                scalar.dma_start(
                    y[mt * P : (mt + 1) * P, :], y_sb[:, mt, :]
                ).then_inc(sY[mt], 16)
            # ACT casts its own final slice then stores it; the sem wait
            # orders the DMA behind the copy's SBUF writes (same-engine
            # issue does NOT imply write completion).
            for n_act, (i, c0, cw) in enumerate(F_ACT, start=1):
                scalar.wait_ge(sPE, n_main + 4 + i)
                scalar.copy(
                    yf_sb[:, c0 : c0 + cw], ps_f[:, c0 : c0 + cw]
                ).then_inc(sCLg, 1)
                scalar.wait_ge(sCLg, n_act)
                scalar.dma_start(
                    yf_dst[r0 : r0 + P, c0 : c0 + cw], yf_sb[:, c0 : c0 + cw]
                ).then_inc(sYscal, 16)
            for mt in range(n_main):
                scalar.wait_ge(sY[mt], 16)
            scalar.wait_ge(sYscal, 16 * len(F_ACT))

        @block.tensor
        def _(tensor):
            def mm01(psum, mt, kt, nt, start, stop):
                return tensor.matmul(
                    psum[:, nt * N_FREE : (nt + 1) * N_FREE],
                    x01_sb[:, kt, mt * P : (mt + 1) * P],
                    w_sb[:, kt, nt * N_FREE : (nt + 1) * N_FREE],
                    start=start,
                    stop=stop,
                    skip_group_check=True,
                )

            def mm(psum, mt, kt, nt, start, stop):
                return tensor.matmul(
                    psum[:, nt * N_FREE : (nt + 1) * N_FREE],
                    x_sb[:, mt - 2, kt * P : (kt + 1) * P],
                    w_sb[:, kt, nt * N_FREE : (nt + 1) * N_FREE],
                    start=start,
                    stop=stop,
                    skip_group_check=True,
                )

            # Pre-warm the PE (HAM un-throttles after ~3.4us of activity)
            # on scratch data while the first DMAs land.
            tensor.wait_ge(sWarm, 1)
            for _ in range(N_WARM):
                tensor.matmul(
                    ps0[:, 0:WARM_N],
                    warm[:, 0:P],
                    warm[:, :],
                    start=True,
                    stop=True,
                    skip_group_check=True,
                )

            # Phase 1: m-tiles 0,1 k-major chasing the W/xq DMA streams.
            for kt in range(KT):
                if kt % 4 == 0:
                    tensor.wait_ge(sXQ[kt // 4], 16)
                tensor.wait_ge(sW[kt], 32)
                last = kt == KT - 1
                for nt in range(NT):
                    m = mm01(ps0, 0, kt, nt, kt == 0, last)
                    if last:
                        m.then_inc(sPE, 1)  # per-nt: mt0 copy pipelines
                for nt in range(NT):
                    m = mm01(ps1, 1, kt, nt, kt == 0, last)
                if last:
                    m.then_inc(sPE, 1)
            # Phase 2 head: m-tile 2 nt-major, gated on m-tile 0's
            # per-slice PSUM copies (no handoff bubble).
            tensor.wait_ge(sX[0], 16)
            for nt in range(NT):
                tensor.wait_ge(sCopy, nt + 1)
                for kt in range(KT):
                    m = mm(ps0, 2, kt, nt, kt == 0, kt == KT - 1)
            m.then_inc(sPE, 1)
            # Phase 2: W resident; stream the remaining m-tiles.
            for mt in range(3, n_main):
                tensor.wait_ge(sX[mt - 2], 16)
                tensor.wait_ge(sCopy, mt + 2)  # psum slot free
                for kt in range(KT):
                    for nt in range(NT):
                        m = mm(psums[mt % 2], mt, kt, nt, kt == 0, kt == KT - 1)
                m.then_inc(sPE, 1)
            # Final tile: col-slice-major so each slice finishes early and
            # its cast+store pipelines behind the remaining matmuls; the
            # last 512 is split in two 256s to shorten the very tail.
            tensor.wait_ge(sXf, 16)
            tensor.wait_ge(sCopy, n_main + 2)
            if ov:
                for kt in range(KTF):
                    tensor.wait_ge(sWf[kt], 16)
            for i, (c0, cw) in enumerate(F_SLICES):
                if ov:
                    # 4 col-tiled jobs per span: col-group q contracts the
                    # group's K-quarter q into psum rows 32q..32q+32; the
                    # four matmuls stream concurrently (own XBUS each).
                    for kl in range(4):
                        for q in range(4):
                            kt = q * 4 + kl
                            m = tensor.matmul(
                                ps_f[32 * q : 32 * (q + 1), c0 : c0 + cw],
                                xf_sb[:, kt * 32 : (kt + 1) * 32],
                                wf_sb[:, kt, c0 : c0 + cw],
                                start=(kl == 0),
                                stop=(kl == 3),
                                skip_group_check=True,
                                tile_position=(0, 32 * q),
                            )
                else:
                    for kt in range(KTF):
                        if i == 0:
                            tensor.wait_ge(sWf[kt], 32)
                        m = tensor.matmul(
                            ps_f[:, c0 : c0 + cw],
                            xf_sb[:, kt * P : (kt + 1) * P],
                            wf_sb[:, kt, c0 : c0 + cw],
                            start=(kt == 0),
                            stop=(kt == KTF - 1),
                            skip_group_check=True,
                        )
                m.then_inc(sPE, 1)

        @block.vector
        def _(vector):
            vector.memset(warm[:, :], 0.25).then_inc(sWarm, 1)
            # m-tile 0 in nt-slices (pipelines with mt1 kt15 + mt2 nt-major)
            for nt in range(NT):
                vector.wait_ge(sPE, nt + 1)
                vector.tensor_copy(
                    y_sb[:, 0, nt * N_FREE : (nt + 1) * N_FREE],
                    ps0[:, nt * N_FREE : (nt + 1) * N_FREE],
                ).then_inc(sCopy, 1)
            for mt in range(1, n_main):
                vector.wait_ge(sPE, mt + 4)
                vector.tensor_copy(
                    y_sb[:, mt, :], psums[mt % 2][:, :]
                ).then_inc(sCopy, 1)
            for i, c0, cw in F_COPIES:
                vector.wait_ge(sPE, n_main + 4 + i)
                vector.tensor_copy(
                    yf_sb[:, c0 : c0 + cw], ps_f[:, c0 : c0 + cw]
                ).then_inc(sCLv, 1)

    return nc


def _route(x, Wg):
    """Host gating: returns token indices per expert and top-1 probs."""
    xf = np.ascontiguousarray(x.reshape(-1, H))
    logits = xf @ Wg  # [T, E] fp32 (min top1-top2 gap ~1e-4)
    idx = logits.argmax(-1)
    m = logits.max(-1, keepdims=True)
    ex = np.exp(logits - m)
    p = (ex[np.arange(len(idx)), idx] / ex.sum(-1)).astype(np.float32)
    return xf, idx, p


def _pack_tiles(xs: np.ndarray, n_tiles: int, k: int, t0_tok: int = 0):
    """tokens [t0_tok + mt*128 + t] -> [n_tiles*128, k] bf16 tiles.

    Row mt*128+p, col kt*128+t  <-  xs[t0_tok + mt*128+t, kt*128+p].
    """
    n = xs.shape[0]
    kt = k // P
    out = np.zeros((n_tiles * P, k), dtype=BF16)
    for mt in range(n_tiles):
        t0, t1 = t0_tok + mt * P, min(t0_tok + (mt + 1) * P, n)
        if t0 >= t1:
            break
        blk = xs[t0:t1].astype(BF16)  # [tc, k]
        tc = t1 - t0
        dst = out[mt * P : (mt + 1) * P].reshape(P, kt, P)  # [p, kt, t]
        dst[:, :, :tc] = blk.reshape(tc, kt, P).transpose(2, 1, 0)
    return out


def _pack_xq(xs: np.ndarray) -> np.ndarray:
    """First 256 tokens -> [512, 1024] quarter-K blocks (see _build_bass)."""
    blk = np.zeros((2 * P, H), dtype=BF16)
    n = min(xs.shape[0], 2 * P)
    blk[:n] = xs[:n].astype(BF16)
    a = blk.reshape(2 * P, KT, P).transpose(1, 2, 0)  # [kt, p, t]
    out = np.empty((4 * P, 4 * 2 * P), dtype=BF16)
    for q in range(4):
        out[q * P : (q + 1) * P] = (
            a[4 * q : 4 * (q + 1)].transpose(1, 0, 2).reshape(P, 4 * 2 * P)
        )
    return out


def _run(inputs, trace=False):
    x = np.asarray(inputs["x"], dtype=np.float32)
    Wg = np.asarray(inputs["Wg"], dtype=np.float32)
    W = np.asarray(inputs["W"], dtype=np.float32)
    b = np.asarray(inputs["b"], dtype=np.float32)

    if trace:
        trace = _ensure_ntff_hook()

    xf, idx, p = _route(x, Wg)
    T = xf.shape[0]

    toks = [np.nonzero(idx == e)[0] for e in range(E)]
    counts = np.array([len(t) for t in toks])

    # Overflow groups: per-expert token chunks beyond CAP, each <= 32
    # (one col-tiled overflow slot per core).  If there are more groups
    # than cores, the smallest go to the host (<=0.1% of FLOPs here).
    groups = []
    for e in range(E):
        o = toks[e][CAP:]
        for i in range(0, len(o), 32):
            groups.append((e, o[i : i + 32]))
    groups.sort(key=lambda g: -len(g[1]))
    host_groups = groups[N_CORES:]
    groups = groups[:N_CORES]

    ov = len(groups) > 0
    if ov:
        n_main = CAP // P
        key = ("OV", n_main)
    else:
        n_main = max(4, int(-(-counts.max() // P)) - 1)
        key = ("A", n_main)
    if key not in _COMPILED:
        _COMPILED[key] = _build_bass(n_main, ov)
    nc = _COMPILED[key]

    MTx = n_main if ov else n_main + 1
    Wbf = [W[e].astype(BF16) for e in range(E)]
    in_maps = []
    for c in range(N_CORES):
        e = c
        te = toks[e][: CAP if ov else None]
        xs = xf[te] * p[te, None]  # fold gate prob into activations
        m = {
            "xq": _pack_xq(xs),
            "xt": _pack_tiles(xs, MTx - 2, H, t0_tok=2 * P),
            "w": Wbf[e],
        }
        if ov:
            if c < len(groups):
                e2, t2 = groups[c]
                xs2 = (xf[t2] * p[t2, None]).astype(BF16)  # [tc<=32, H]
                tc = len(t2)
                xt2 = np.zeros((P, KT * 32), dtype=BF16)
                # col block kt holds k-rows kt*128..: xt2[p, kt*32+t]
                a = xs2.reshape(tc, KT, P).transpose(1, 2, 0)  # [kt, p, t]
                xt2.reshape(P, KT, 32)[:, :, :tc] = a.transpose(1, 0, 2)
                m["xt2"] = xt2
                m["w2"] = Wbf[e2]
            else:
                m["xt2"] = np.zeros((P, KT * 32), dtype=BF16)
                m["w2"] = np.zeros((H, H), dtype=BF16)
        in_maps.append(m)

    res = run_bass_kernel_spmd(
        nc,
        in_maps,
        core_ids=list(range(N_CORES)),
        trace=trace,
        trace_cores=list(range(N_CORES)) if trace else None,
    )

    out = np.empty((T, H), dtype=np.float32)
    for e in range(E):
        te = toks[e][: CAP if ov else None]
        ye = res.results[e]["y"][: len(te)].astype(np.float32)
        if np.any(b[e]):
            ye = ye + p[te, None] * b[e]
        out[te] = ye
    if ov:
        for g, (e2, t2) in enumerate(groups):
            tc = len(t2)
            y2 = res.results[g]["y2"]
            ye = sum(
                y2[32 * q : 32 * q + tc].astype(np.float32) for q in range(4)
            )
            if np.any(b[e2]):
                ye = ye + p[t2, None] * b[e2]
            out[t2] = ye
        for e2, t2 in host_groups:
            ye = (p[t2, None] * (xf[t2] @ W[e2])).astype(np.float32)
            if np.any(b[e2]):
                ye = ye + p[t2, None] * b[e2]
            out[t2] = ye
    return out.reshape(B, S, H), res


def kernel(**inputs) -> np.ndarray:
    out, _ = _run(inputs, trace=os.environ.get("MOE_TRACE", "0") == "1")
    return out


def run_traced(inputs):
    """For test.py: returns (output, BassKernelResults with exec_time_ns)."""
    return _run(inputs, trace=True)
